# revision 14
# baseline (speedup 1.0000x reference)
# Fused attention block (LeViT-style) for Trainium2, 8 NeuronCores, data-parallel over batch.
#
# reference computation (B=16, N=784, DIM=512, H=8, KD=64, VD=256):
#   qkv = BN(x @ qkv_w.T); split q,k,v per head
#   attn = softmax(q @ k.T * KD**-0.5 + attention_biases[:, bias_idxs])
#   out  = BN(silu(attn @ v reshaped) @ proj_w.T)
#
# Strategy:
#  - batch-parallel: 2 batches per core, weights/bias tables replicated, no collectives
#  - BN folded into weights on host; softmax scale folded into q weights
#  - all matmul operands bf16 (PSUM accumulation fp32), softmax pipeline fp32
#  - scores computed transposed (S^T[j,i]); bias table is symmetric so bias adds unchanged
#  - softmax denominator from an extra ones-column in v (col 256 of each head block)
#  - unstabilized softmax (scores empirically bounded ~|10|, exp is safe in fp32)

import numpy as np
import ml_dtypes

B, N, DIM = 16, 784, 512
H, KD, VD = 8, 64, 256
RES = 28
EPS = 1e-5
SCALE = KD ** -0.5
NCORES = 8
BL = B // NCORES          # batches per core
VDA = VD + 1              # v head block with ones column
OVW = H * VDA             # 2056
NJP = 896                 # padded j extent (7 * 128)

# t/j chunking over N=784: six 128-chunks + one 16-chunk
CHUNKS = [(i * 128, min(128, N - i * 128)) for i in range((N + 127) // 128)]
ITILES = [(0, 512), (512, N - 512)]   # free-dim tiles for 784 (<=512 per PSUM bank)

_CACHE = {}


def _build_nc():
    from contextlib import ExitStack
    import concourse.bacc as bacc
    import concourse.tile as tile
    from concourse import mybir

    bf = mybir.dt.bfloat16
    f32 = mybir.dt.float32
    AF = mybir.ActivationFunctionType
    ADD = mybir.AluOpType.add

    nc = bacc.Bacc("TRN2", target_bir_lowering=False, debug=False)

    xT = nc.dram_tensor("xT", [BL, DIM, N], bf, kind="ExternalInput").ap()
    wqk = nc.dram_tensor("wqk", [128, 4, 1024], bf, kind="ExternalInput").ap()
    wv = nc.dram_tensor("wv", [128, 4, OVW], bf, kind="ExternalInput").ap()
    wp = nc.dram_tensor("wp", [128, 16, DIM], bf, kind="ExternalInput").ap()
    bqk = nc.dram_tensor("bqk", [128, 8], f32, kind="ExternalInput").ap()
    bv = nc.dram_tensor("bv", [OVW], bf, kind="ExternalInput").ap()
    bp = nc.dram_tensor("bp", [1, DIM], bf, kind="ExternalInput").ap()
    biast = nc.dram_tensor("biast", [H, NJP, N], bf, kind="ExternalInput").ap()
    ones = nc.dram_tensor("ones", [1, 128], bf, kind="ExternalInput").ap()
    ident = nc.dram_tensor("ident", [128, 128], bf, kind="ExternalInput").ap()
    out = nc.dram_tensor("out", [BL, N, DIM], f32, kind="ExternalOutput").ap()

    import concourse.bass as bass

    with ExitStack() as ctx:
        tc = ctx.enter_context(tile.TileContext(nc))
        consts = ctx.enter_context(tc.tile_pool(name="consts", bufs=1))
        xpool = ctx.enter_context(tc.tile_pool(name="xpool", bufs=1))
        qkpool = ctx.enter_context(tc.tile_pool(name="qkpool", bufs=1))
        vpool = ctx.enter_context(tc.tile_pool(name="vpool", bufs=1))
        silupool = ctx.enter_context(tc.tile_pool(name="silupool", bufs=1))
        biaspool = ctx.enter_context(tc.tile_pool(name="biaspool", bufs=2))
        ppool = ctx.enter_context(tc.tile_pool(name="ppool", bufs=2))
        smalls = ctx.enter_context(tc.tile_pool(name="smalls", bufs=4))
        fpool = ctx.enter_context(tc.tile_pool(name="fpool", bufs=2))
        tpool = ctx.enter_context(tc.tile_pool(name="tpool", bufs=3))
        pssm = ctx.enter_context(tc.tile_pool(name="pssm", bufs=6, space="PSUM"))
        pstp = ctx.enter_context(tc.tile_pool(name="pstp", bufs=2, space="PSUM"))

        # ---- constants ----
        wqk_sb = consts.tile([128, 4, 1024], bf)
        nc.sync.dma_start(out=wqk_sb, in_=wqk)
        wv_sb = consts.tile([128, 4, OVW], bf)
        nc.sync.dma_start(out=wv_sb, in_=wv)
        wp_sb = consts.tile([128, 16, DIM], bf)
        nc.sync.dma_start(out=wp_sb, in_=wp)
        bqk_sb = consts.tile([128, 8], f32)
        nc.sync.dma_start(out=bqk_sb, in_=bqk)
        bp_sb = consts.tile([1, DIM], bf)
        nc.sync.dma_start(out=bp_sb, in_=bp)
        ones_sb = consts.tile([1, 128], bf)
        nc.sync.dma_start(out=ones_sb, in_=ones)
        ident_sb = consts.tile([128, 128], bf)
        nc.sync.dma_start(out=ident_sb, in_=ident)
        # v-bias as a single row (rank-1 PE matmul adds it into PSUM)
        bv_sb = consts.tile([1, OVW], bf)
        nc.sync.dma_start(out=bv_sb, in_=bv[None, :])

        for b in range(BL):
            # ---- load xT[b]: [512, 784] -> [128, cc, 784] ----
            xT_sb = xpool.tile([128, 4, N], bf)
            xin = bass.AP(
                tensor=xT.tensor,
                offset=xT.offset + b * DIM * N,
                ap=[[N, 128], [128 * N, 4], [1, N]],
            )
            nc.sync.dma_start(out=xT_sb, in_=xin)

            # ---- pass A: qkT[o, t] for all heads (o-chunks 0-3 = q, 4-7 = k) ----
            qk_sb = qkpool.tile([128, 8, N], bf)
            for oc in range(8):
                for (i0, isz) in ITILES:
                    ps = pssm.tile([128, 512], f32, tag="small")
                    for cc in range(4):
                        nc.tensor.matmul(
                            ps[:, :isz],
                            lhsT=wqk_sb[:, cc, oc * 128:(oc + 1) * 128],
                            rhs=xT_sb[:, cc, i0:i0 + isz],
                            start=(cc == 0),
                            stop=(cc == 3),
                        )
                    nc.vector.tensor_scalar_add(
                        out=qk_sb[:, oc, i0:i0 + isz], in0=ps[:, :isz],
                        scalar1=bqk_sb[:, oc:oc + 1],
                    )

            # ---- pass B: v[t, h*257+d'] with ones cols via bias add ----
            v_sb = vpool.tile([128, 7, OVW], bf)
            for tc_i, (t0, tsz) in enumerate(CHUNKS):
                for ovt in range(8):
                    o0 = ovt * VDA
                    ps = pssm.tile([128, 512], f32, tag="small")
                    nc.tensor.matmul(
                        ps[:tsz, :VDA],
                        lhsT=ones_sb[0:1, :tsz],
                        rhs=bv_sb[0:1, o0:o0 + VDA],
                        start=True,
                        stop=False,
                    )
                    for cc in range(4):
                        nc.tensor.matmul(
                            ps[:tsz, :VDA],
                            lhsT=xT_sb[:, cc, t0:t0 + tsz],
                            rhs=wv_sb[:, cc, o0:o0 + VDA],
                            start=False,
                            stop=(cc == 3),
                        )
                    nc.scalar.activation(
                        out=v_sb[:tsz, tc_i, o0:o0 + VDA],
                        in_=ps[:tsz, :VDA], func=AF.Copy,
                    )

            silu_sb = silupool.tile([128, 7, H * VD], bf)

            # ---- per head: S^T -> exp -> AV -> silu ----
            for h in range(H):
                base = (h % 2) * 64
                qoc = h // 2
                koc = 4 + h // 2

                bias_sb = biaspool.tile([128, 7, N], bf)
                bin_ = bass.AP(
                    tensor=biast.tensor,
                    offset=biast.offset + h * NJP * N,
                    ap=[[N, 128], [128 * N, 7], [1, N]],
                )
                nc.sync.dma_start(out=bias_sb, in_=bin_)

                p_sb = ppool.tile([128, 7, N], bf)
                for jc, (j0, jsz) in enumerate(CHUNKS):
                    for (i0, isz) in ITILES:
                        ps = pssm.tile([128, 512], f32, tag="small")
                        nc.tensor.matmul(
                            ps[:jsz, :isz],
                            lhsT=qk_sb[base:base + 64, koc, j0:j0 + jsz],
                            rhs=qk_sb[base:base + 64, qoc, i0:i0 + isz],
                            start=True,
                            stop=True,
                        )
                        # bias add in place in PSUM, then exp PSUM -> SBUF bf16
                        nc.vector.tensor_tensor(
                            out=ps[:jsz, :isz], in0=ps[:jsz, :isz],
                            in1=bias_sb[:jsz, jc, i0:i0 + isz], op=ADD,
                        )
                        nc.scalar.activation(
                            out=p_sb[:jsz, jc, i0:i0 + isz], in_=ps[:jsz, :isz],
                            func=AF.Exp,
                        )

                for ic, (i0, isz) in enumerate(CHUNKS):
                    ps = pssm.tile([128, 512], f32, tag="small")
                    for jc, (j0, jsz) in enumerate(CHUNKS):
                        nc.tensor.matmul(
                            ps[:isz, :VDA],
                            lhsT=p_sb[:jsz, jc, i0:i0 + isz],
                            rhs=v_sb[:jsz, jc, h * VDA:(h + 1) * VDA],
                            start=(jc == 0),
                            stop=(jc == 6),
                        )
                    rs = smalls.tile([128, 1], f32)
                    nc.vector.reciprocal(out=rs[:isz], in_=ps[:isz, VD:VDA])
                    # normalized pre-silu values (silu applied in bulk at batch end)
                    nc.vector.tensor_scalar_mul(
                        out=silu_sb[:isz, ic, h * VD:(h + 1) * VD],
                        in0=ps[:isz, :VD], scalar1=rs[:isz, 0:1],
                    )

            # ---- proj: transpose silu chunks, accumulate over 16 v-chunks ----
            for tc_i, (t0, tsz) in enumerate(CHUNKS):
                nc.scalar.activation(
                    out=silu_sb[:tsz, tc_i, :], in_=silu_sb[:tsz, tc_i, :],
                    func=AF.Silu,
                )
                psf = pssm.tile([128, 512], f32, tag="small")
                nc.tensor.matmul(
                    psf[:tsz, :],
                    lhsT=ones_sb[0:1, :tsz],
                    rhs=bp_sb[0:1, :],
                    start=True,
                    stop=False,
                )
                for vp in range(8):
                    pst = pstp.tile([128, 2, 128], bf, tag="tp")
                    for k in range(2):
                        vc = vp * 2 + k
                        nc.tensor.transpose(
                            pst[:, k, :tsz],
                            silu_sb[:tsz, tc_i, vc * 128:(vc + 1) * 128],
                            ident_sb[:tsz, :tsz],
                        )
                    st = tpool.tile([128, 2, 128], bf)
                    nc.scalar.copy(out=st[:, :, :tsz], in_=pst[:, :, :tsz])
                    for k in range(2):
                        vc = vp * 2 + k
                        nc.tensor.matmul(
                            psf[:tsz, :],
                            lhsT=st[:, k, :tsz],
                            rhs=wp_sb[:, vc, :],
                            start=False,
                            stop=(vc == 15),
                        )
                fo = fpool.tile([128, DIM], f32)
                nc.scalar.activation(out=fo[:tsz], in_=psf[:tsz], func=AF.Copy)
                nc.sync.dma_start(out=out[b, t0:t0 + tsz, :], in_=fo[:tsz])

    nc.finalize()
    return nc


def _prep(inputs):
    bf16 = ml_dtypes.bfloat16
    f32 = np.float32
    inputs = {k: np.asarray(v) for k, v in inputs.items()}

    s_qkv = (inputs["qkv_gamma"] / np.sqrt(inputs["qkv_var"] + EPS)).astype(f32)
    b_qkv = (inputs["qkv_beta"] - inputs["qkv_mean"] * s_qkv).astype(f32)
    w_fold = (inputs["qkv_w"] * s_qkv[:, None]).astype(f32)

    rows = np.arange((2 * KD + VD) * H).reshape(H, 2 * KD + VD)
    q_rows = rows[:, :KD].ravel()
    k_rows = rows[:, KD:2 * KD].ravel()
    v_rows = rows[:, 2 * KD:].ravel()

    wq = w_fold[q_rows] * SCALE
    bq = b_qkv[q_rows] * SCALE
    wk = w_fold[k_rows]
    bk = b_qkv[k_rows]
    wvm = w_fold[v_rows]
    bvm = b_qkv[v_rows]

    # wqk: [c, o] with o = [q(512), k(512)] -> [128, cc, 1024]
    wqkT = np.concatenate([wq, wk], axis=0).T.astype(bf16)          # [512, 1024]
    wqk_t = np.ascontiguousarray(wqkT.reshape(4, 128, 1024).transpose(1, 0, 2))
    bqk_t = np.concatenate([bq, bk]).reshape(8, 128).T.astype(f32)  # [128, 8]
    bqk_t = np.ascontiguousarray(bqk_t)

    # wv augmented: per-head 257-col blocks, col 256 zero (ones come from bias)
    wv_aug = np.zeros((DIM, OVW), dtype=f32)
    bv_aug = np.zeros(OVW, dtype=f32)
    for h in range(H):
        wv_aug[:, h * VDA:h * VDA + VD] = wvm[h * VD:(h + 1) * VD].T
        bv_aug[h * VDA:h * VDA + VD] = bvm[h * VD:(h + 1) * VD]
        bv_aug[h * VDA + VD] = 1.0
    wv_t = np.ascontiguousarray(wv_aug.astype(bf16).reshape(4, 128, OVW).transpose(1, 0, 2))
    bv_t = bv_aug.astype(bf16)

    s_p = (inputs["proj_gamma"] / np.sqrt(inputs["proj_var"] + EPS)).astype(f32)
    b_p = (inputs["proj_beta"] - inputs["proj_mean"] * s_p).astype(f32)
    wp_fold = (inputs["proj_w"] * s_p[:, None]).astype(f32)          # [512, 2048]
    wp_t = np.ascontiguousarray(
        wp_fold.T.astype(bf16).reshape(16, 128, DIM).transpose(1, 0, 2)
    )
    bp_t = b_p.astype(bf16)[None, :]

    bias_full = inputs["attention_biases"][:, inputs["bias_idxs"]].astype(f32)  # [H, N, N]
    biast = np.zeros((H, NJP, N), dtype=bf16)
    biast[:, :N, :] = bias_full.astype(bf16)

    xT = inputs["x"].transpose(0, 2, 1).astype(bf16)                 # [B, 512, 784]

    shared = {
        "wqk": wqk_t, "wv": wv_t, "wp": wp_t, "bqk": bqk_t,
        "bv": bv_t, "bp": np.ascontiguousarray(bp_t), "biast": biast,
        "ones": np.ones((1, 128), dtype=bf16),
        "ident": np.eye(128, dtype=np.float32).astype(bf16),
    }
    in_maps = []
    for c in range(NCORES):
        m = dict(shared)
        m["xT"] = np.ascontiguousarray(xT[c * BL:(c + 1) * BL])
        in_maps.append(m)
    return in_maps


def kernel(trace=False, **inputs):
    from concourse import bass_utils

    if "nc" not in _CACHE:
        _CACHE["nc"] = _build_nc()
    nc = _CACHE["nc"]

    in_maps = _prep(inputs)
    res = bass_utils.run_bass_kernel_spmd(
        nc, in_maps, core_ids=list(range(NCORES)), trace=trace,
    )
    out = np.concatenate([r["out"] for r in res.results], axis=0)
    if trace:
        return out.astype(np.float32), res
    return out.astype(np.float32)
